# revision 20
# baseline (speedup 1.0000x reference)
"""MixtureOfDepths router kernel for 8 Trainium2 NeuronCores.

Problem (hardcoded shapes): hidden_states (4, 8192, 4096) f32, router weight
w (4096,) f32, bias b () f32.
  logits = hidden_states @ w + b        (4, 8192)
  weights = sigmoid(logits)
  k = 4096; threshold = k-th largest weight per batch row
  mask = weights >= threshold

Sharding: core c handles batch c//2, sequence half c%2 -> a (4096, 4096)
slice (64 MiB).  Per core: 32 tiles of [128 tokens x 4096 hidden], one DVE
tensor_tensor_reduce (mult + add-reduce, bias as init) per tile -> logits
[128, 32]; ACT sigmoid -> weights.  Pairwise AllGather (cores 2b, 2b+1)
shares the batch's 8192 weights.  The k-th-largest threshold is found
exactly via 128-way radix bisection over the sigmoid outputs' int32 bit
patterns (positive floats are order-isomorphic to their bits): 5 rounds with
steps 2^23, 2^16, 2^9, 2^2, 1; each round counts elements >= 128
per-partition candidates with a single tensor_scalar(is_ge, accum_out) op,
then updates the base arithmetically (no branches).  Ties handled exactly
like the reference (mask = w >= kth value).
"""

import sys

if "/opt/trn_rl_repo" not in sys.path:
    sys.path.insert(0, "/opt/trn_rl_repo")

from contextlib import ExitStack

import numpy as np

import concourse.bass as bass  # noqa: F401  (bass types via bacc)
import concourse.tile as tile
from concourse import bacc, mybir
from concourse import bass_isa
from concourse import bass2jax
from concourse import mybir as _mb

N_CORES = 8
BATCH = 4
SEQ = 8192
HIDDEN = 4096
K = SEQ // 2  # 4096

# Radix bisection steps covering sigmoid bit range [0, 2^30): 23+... bits.
BISECT_STEPS = [1 << 23, 1 << 16, 1 << 9, 1 << 2, 1]


def build(n_cores=N_CORES, tok=SEQ // 2, hidden=HIDDEN, k=K, pair_groups=None,
          fake_gather=False):
    """Build the SPMD bass module. Each core: tok tokens x hidden dots,
    sigmoid, pairwise allgather (2*tok weights), exact kth-largest bisect,
    mask output."""
    f32, i32, u8 = mybir.dt.float32, mybir.dt.int32, mybir.dt.uint8
    ntile = tok // 128
    assert tok % 128 == 0
    if pair_groups is None:
        pair_groups = [[2 * i, 2 * i + 1] for i in range(n_cores // 2)]

    nc = bacc.Bacc("TRN2", target_bir_lowering=False, debug=False,
                   num_devices=n_cores)

    nsteps = len(BISECT_STEPS)
    hs = nc.dram_tensor("hs", [tok, hidden], f32, kind="ExternalInput").ap()
    w2 = nc.dram_tensor("w2", [128, hidden], f32, kind="ExternalInput").ap()
    bias2 = nc.dram_tensor("bias2", [128, 1], f32, kind="ExternalInput").ap()
    # iosc[p, r] = p * BISECT_STEPS[r]  (host-precomputed)
    iosc = nc.dram_tensor("iosc", [128, nsteps], i32, kind="ExternalInput").ap()
    wout = nc.dram_tensor("wout", [128, ntile], f32, kind="ExternalOutput").ap()
    mout = nc.dram_tensor("mout", [128, ntile], u8, kind="ExternalOutput").ap()

    # token t = p * ntile + n  ->  partition p, tile-slot n
    hs3 = hs.rearrange("(p n) d -> p n d", p=128)

    with tile.TileContext(nc) as tc, ExitStack() as ctx:
        consts = ctx.enter_context(tc.tile_pool(name="consts", bufs=1))
        hpool = ctx.enter_context(tc.tile_pool(name="hid", bufs=3))
        spool = ctx.enter_context(tc.tile_pool(name="big", bufs=1))
        small = ctx.enter_context(tc.tile_pool(name="small", bufs=1))
        dram = ctx.enter_context(tc.tile_pool(name="dram", bufs=1, space="DRAM"))

        wb = consts.tile([128, hidden], f32)
        nc.sync.dma_start(out=wb[:], in_=w2[:])
        bb = consts.tile([128, 1], f32)
        nc.sync.dma_start(out=bb[:], in_=bias2[:])
        io = consts.tile([128, nsteps], i32)
        nc.sync.dma_start(out=io[:], in_=iosc[:])

        logits = small.tile([128, ntile], f32, tag="logits")

        for i in range(ntile):
            ht = hpool.tile([128, hidden], f32, tag="ht")
            nc.sync.dma_start(out=ht[:], in_=hs3[:, i, :])
            sc = spool.tile([128, hidden], f32, tag="sc")
            # dot(ht[p, :], w) via scalar_tensor_tensor + sum-accumulator.
            # (tensor_tensor_reduce crashes at runtime in this environment;
            # the InstTensorScalarPtr family is HW-verified.)
            nc.vector.scalar_tensor_tensor(
                out=sc[:], in0=ht[:], scalar=1.0, in1=wb[:],
                op0=mybir.AluOpType.mult, op1=mybir.AluOpType.mult,
                accum_out=logits[:, i:i + 1])

        # sigmoid(logits + bias): the router bias folds into ACT's bias.
        wsig = small.tile([128, ntile], f32, tag="wsig")
        nc.scalar.activation(out=wsig[:], in_=logits[:],
                             func=mybir.ActivationFunctionType.Sigmoid,
                             bias=bb[:])
        nc.sync.dma_start(out=wout[:], in_=wsig[:])

        # ---- pairwise allgather of this core's weights ----
        gin = dram.tile([128, ntile], f32)
        nc.sync.dma_start(out=gin[:], in_=wsig[:])
        gout = dram.tile([1, 2 * tok], f32)
        if fake_gather:
            # single-core timeline-sim stand-in for the pairwise AllGather
            g2 = gout[:].rearrange("a (h t) -> a h t", h=2)
            nc.sync.dma_start(out=g2[:, 0, :], in_=gin.opt())
            nc.sync.dma_start(out=g2[:, 1, :], in_=gin.opt())
        else:
            nc.gpsimd.collective_compute(
                "AllGather",
                mybir.AluOpType.bypass,
                replica_groups=pair_groups,
                ins=[gin.opt()],
                outs=[gout.opt()],
            )
        gs = spool.tile([1, 2 * tok], f32, tag="gs")
        nc.sync.dma_start(out=gs[:], in_=gout[:])
        wall = spool.tile([128, 2 * tok], f32, tag="wall")
        nc.gpsimd.partition_broadcast(wall[:], gs[:], channels=128)

        # ---- exact kth-largest via 128-way radix bisection on bit space ----
        base = small.tile([128, 1], i32, tag="base0")
        nc.vector.memset(base[:], 0)
        base_alt = small.tile([128, 1], i32, tag="base1")
        cnt = small.tile([128, 1], f32, tag="cnt")
        flag = small.tile([128, 1], f32, tag="flag")
        sumf = small.tile([128, 1], f32, tag="sumf")
        delta = small.tile([128, 1], i32, tag="delta")
        csc = spool.tile([128, 2 * tok], f32, tag="csc")

        for r, s in enumerate(BISECT_STEPS):
            cand = small.tile([128, 1], i32, tag="cand")
            # cand[p] = p * s + base[p].  MUST be on gpsimd: the DVE ALU is
            # fp32-internal, so int32 adds at magnitude 2^30 round to 64s
            # (HW-verified).  Q7 int32 adds are exact.
            nc.gpsimd.tensor_add(cand[:], io[:, r:r + 1], base[:])
            # cnt[p] = sum_j (wall[j] >= float_view(cand[p]))
            # candidate bit patterns are all valid non-negative f32 < 1.0, and
            # the weights are sigmoid outputs in (0,1), so float compare ==
            # bit-int compare (no denormal/negative pitfalls near threshold).
            nc.vector.tensor_scalar(
                out=csc[:], in0=wall[:], scalar1=cand[:].bitcast(f32),
                scalar2=None, op0=mybir.AluOpType.is_ge,
                op1=mybir.AluOpType.add, accum_out=cnt[:])
            # flag[p] = cnt[p] >= k
            nc.vector.tensor_scalar(
                out=flag[:], in0=cnt[:], scalar1=float(k), scalar2=None,
                op0=mybir.AluOpType.is_ge)
            # sumf = sum_p flag[p]  (same value on every partition)
            nc.gpsimd.partition_all_reduce(
                sumf[:], flag[:], channels=128,
                reduce_op=bass_isa.ReduceOp.add)
            # delta = (sumf - 1) * s, computed in f32 (exact: |delta| <=
            # 127 * 2^23 is a 7-bit mantissa times a power of two) with
            # conversion to int32 on the write.
            nc.vector.tensor_scalar(
                out=delta[:], in0=sumf[:], scalar1=1.0, scalar2=float(s),
                op0=mybir.AluOpType.subtract, op1=mybir.AluOpType.mult)
            # base += delta — gpsimd for exact int32 addition (see above).
            nc.gpsimd.tensor_add(base_alt[:], delta[:], base[:])
            base, base_alt = base_alt, base

        # ---- mask: own weights >= threshold (exact kth-largest value) ----
        mask = small.tile([128, ntile], u8, tag="mask")
        nc.vector.tensor_scalar(
            out=mask[:], in0=wsig[:], scalar1=base[:].bitcast(f32),
            scalar2=None, op0=mybir.AluOpType.is_ge)
        nc.sync.dma_start(out=mout[:], in_=mask[:])

    nc.compile()
    return nc


class Runner:
    """Executes a built Bass module on the 8 axon NeuronCores via PJRT,
    building the sharded jit executable once and reusing it (the stock
    run_bass_kernel_spmd re-jits on every call)."""

    def __init__(self, nc, n_cores=N_CORES):
        import jax
        from jax.sharding import Mesh, PartitionSpec
        from jax.experimental.shard_map import shard_map

        bass2jax.install_neuronx_cc_hook()
        self.n_cores = n_cores
        partition_name = (nc.partition_id_tensor.name
                          if nc.partition_id_tensor else None)
        in_names, out_names, out_avals, zero_outs = [], [], [], []
        for alloc in nc.m.functions[0].allocations:
            if not isinstance(alloc, _mb.MemoryLocationSet):
                continue
            name = alloc.memorylocations[0].name
            if alloc.kind == "ExternalInput":
                if name != partition_name:
                    in_names.append(name)
            elif alloc.kind == "ExternalOutput":
                shape = tuple(alloc.tensor_shape)
                dtype = _mb.dt.np(alloc.dtype)
                out_names.append(name)
                out_avals.append(jax.core.ShapedArray(shape, dtype))
                zero_outs.append(np.zeros(shape, dtype))
        self.in_names, self.out_names = list(in_names), out_names
        self.out_avals, self.zero_outs = out_avals, zero_outs
        n_params, n_outs = len(in_names), len(out_avals)
        self.n_params = n_params
        all_names = in_names + out_names
        if partition_name is not None:
            all_names = all_names + [partition_name]

        def _body(*args):
            operands = list(args)
            if partition_name is not None:
                operands.append(bass2jax.partition_id_tensor())
            return tuple(bass2jax._bass_exec_p.bind(
                *operands,
                out_avals=tuple(out_avals),
                in_names=tuple(all_names),
                out_names=tuple(out_names),
                lowering_input_output_aliases=(),
                sim_require_finite=True,
                sim_require_nnan=True,
                nc=nc,
            ))

        devices = jax.devices()[:n_cores]
        self.mesh = Mesh(np.asarray(devices), ("core",))
        self.pspec = PartitionSpec("core")
        in_specs = (self.pspec,) * (n_params + n_outs)
        out_specs = (self.pspec,) * n_outs
        self.sharded = jax.jit(
            shard_map(_body, mesh=self.mesh, in_specs=in_specs,
                      out_specs=out_specs, check_rep=False),
            donate_argnums=tuple(range(n_params, n_params + n_outs)),
            keep_unused=True)

    def concat_inputs(self, in_maps):
        return [np.concatenate([np.asarray(in_maps[c][nm])
                                for c in range(self.n_cores)], axis=0)
                for nm in self.in_names]

    def fresh_zeros(self):
        return [np.zeros((self.n_cores * z.shape[0], *z.shape[1:]), z.dtype)
                for z in self.zero_outs]

    def call(self, concat_in):
        """concat_in: list of (n_cores*dim0, ...) arrays (host or device)."""
        return self.sharded(*concat_in, *self.fresh_zeros())

    def run(self, in_maps):
        out_arrs = self.call(self.concat_inputs(in_maps))
        return [
            {nm: np.asarray(out_arrs[i]).reshape(
                self.n_cores, *self.out_avals[i].shape)[c]
             for i, nm in enumerate(self.out_names)}
            for c in range(self.n_cores)
        ]


_NC_CACHE = {}


def _get_nc():
    if "full" not in _NC_CACHE:
        _NC_CACHE["full"] = build()
    return _NC_CACHE["full"]


def _get_runner():
    if "runner" not in _NC_CACHE:
        _NC_CACHE["runner"] = Runner(_get_nc())
    return _NC_CACHE["runner"]


def make_in_maps(hidden_states, w, b, n_cores=N_CORES, tok=SEQ // 2):
    hs = np.asarray(hidden_states, dtype=np.float32)
    wv = np.asarray(w, dtype=np.float32).reshape(-1)
    hidden = wv.shape[0]
    w2 = np.ascontiguousarray(np.broadcast_to(wv[None, :], (128, hidden)))
    bias2 = np.full((128, 1), np.float32(b), dtype=np.float32)
    iosc = (np.arange(128, dtype=np.int64)[:, None]
            * np.asarray(BISECT_STEPS, dtype=np.int64)[None, :])
    iosc = iosc.astype(np.int32)
    in_maps = []
    for c in range(n_cores):
        bb, h = c // 2, c % 2
        shard = np.ascontiguousarray(hs[bb, h * tok:(h + 1) * tok, :])
        in_maps.append({"hs": shard, "w2": w2, "bias2": bias2, "iosc": iosc})
    return in_maps


def assemble(results, n_cores=N_CORES, tok=SEQ // 2):
    weights = np.empty((BATCH, SEQ), dtype=np.float32)
    mask = np.empty((BATCH, SEQ), dtype=bool)
    for c in range(n_cores):
        bb, h = c // 2, c % 2
        weights[bb, h * tok:(h + 1) * tok] = results[c]["wout"].reshape(-1)
        mask[bb, h * tok:(h + 1) * tok] = results[c]["mout"].reshape(-1) != 0
    return weights, mask


def kernel(hidden_states, w, b):
    runner = _get_runner()
    in_maps = make_in_maps(hidden_states, w, b)
    return assemble(runner.run(in_maps))


# revision 24
# speedup vs baseline: 13.6101x; 13.6101x over previous
"""MixtureOfDepths router kernel for 8 Trainium2 NeuronCores.

Problem (hardcoded shapes): hidden_states (4, 8192, 4096) f32, router weight
w (4096,) f32, bias b () f32.
  logits = hidden_states @ w + b        (4, 8192)
  weights = sigmoid(logits)
  k = 4096; threshold = k-th largest weight per batch row
  mask = weights >= threshold

Sharding: core c handles batch c//2, sequence half c%2 -> a (4096, 4096)
slice (64 MiB).  Per core: 32 tiles of [128 tokens x 4096 hidden], one DVE
tensor_tensor_reduce (mult + add-reduce, bias as init) per tile -> logits
[128, 32]; ACT sigmoid -> weights.  Pairwise AllGather (cores 2b, 2b+1)
shares the batch's 8192 weights.  The k-th-largest threshold is found
exactly via 128-way radix bisection over the sigmoid outputs' int32 bit
patterns (positive floats are order-isomorphic to their bits): 5 rounds with
steps 2^23, 2^16, 2^9, 2^2, 1; each round counts elements >= 128
per-partition candidates with a single tensor_scalar(is_ge, accum_out) op,
then updates the base arithmetically (no branches).  Ties handled exactly
like the reference (mask = w >= kth value).
"""

import sys

if "/opt/trn_rl_repo" not in sys.path:
    sys.path.insert(0, "/opt/trn_rl_repo")

from contextlib import ExitStack

import numpy as np

import concourse.bass as bass  # noqa: F401  (bass types via bacc)
import concourse.tile as tile
from concourse import bacc, mybir
from concourse import bass_isa
from concourse import bass2jax
from concourse import mybir as _mb

N_CORES = 8
BATCH = 4
SEQ = 8192
HIDDEN = 4096
K = SEQ // 2  # 4096

# Radix bisection steps covering sigmoid bit range [0, 2^30): 23+... bits.
BISECT_STEPS = [1 << 23, 1 << 16, 1 << 9, 1 << 2, 1]


def build(n_cores=N_CORES, tok=SEQ // 2, hidden=HIDDEN, k=K, pair_groups=None,
          fake_gather=False, hbufs=3, delta_on_q7=False, dma_bcast=False):
    """Build the SPMD bass module. Each core: tok tokens x hidden dots,
    sigmoid, pairwise allgather (2*tok weights), exact kth-largest bisect,
    mask output."""
    f32, i32, u8 = mybir.dt.float32, mybir.dt.int32, mybir.dt.uint8
    ntile = tok // 128
    assert tok % 128 == 0
    if pair_groups is None:
        pair_groups = [[2 * i, 2 * i + 1] for i in range(n_cores // 2)]

    nc = bacc.Bacc("TRN2", target_bir_lowering=False, debug=False,
                   num_devices=n_cores)

    nsteps = len(BISECT_STEPS)
    hs = nc.dram_tensor("hs", [tok, hidden], f32, kind="ExternalInput").ap()
    w2 = nc.dram_tensor("w2", [128, hidden], f32, kind="ExternalInput").ap()
    bias2 = nc.dram_tensor("bias2", [128, 1], f32, kind="ExternalInput").ap()
    # iosc[p, r] = p * BISECT_STEPS[r]  (host-precomputed)
    iosc = nc.dram_tensor("iosc", [128, nsteps], i32, kind="ExternalInput").ap()
    wout = nc.dram_tensor("wout", [128, ntile], f32, kind="ExternalOutput").ap()
    mout = nc.dram_tensor("mout", [128, ntile], u8, kind="ExternalOutput").ap()

    # token t = p * ntile + n  ->  partition p, tile-slot n
    hs3 = hs.rearrange("(p n) d -> p n d", p=128)

    with tile.TileContext(nc) as tc, ExitStack() as ctx:
        consts = ctx.enter_context(tc.tile_pool(name="consts", bufs=1))
        hpool = ctx.enter_context(tc.tile_pool(name="hid", bufs=hbufs))
        spool = ctx.enter_context(tc.tile_pool(name="big", bufs=1))
        small = ctx.enter_context(tc.tile_pool(name="small", bufs=1))
        dram = ctx.enter_context(tc.tile_pool(name="dram", bufs=1, space="DRAM"))

        wb = consts.tile([128, hidden], f32)
        nc.sync.dma_start(out=wb[:], in_=w2[:])
        bb = consts.tile([128, 1], f32)
        nc.sync.dma_start(out=bb[:], in_=bias2[:])
        io = consts.tile([128, nsteps], i32)
        nc.sync.dma_start(out=io[:], in_=iosc[:])

        logits = small.tile([128, ntile], f32, tag="logits")

        for i in range(ntile):
            ht = hpool.tile([128, hidden], f32, tag="ht")
            nc.sync.dma_start(out=ht[:], in_=hs3[:, i, :])
            sc = spool.tile([128, hidden], f32, tag="sc")
            # dot(ht[p, :], w) via scalar_tensor_tensor + sum-accumulator.
            # (tensor_tensor_reduce crashes at runtime in this environment;
            # the InstTensorScalarPtr family is HW-verified.)
            nc.vector.scalar_tensor_tensor(
                out=sc[:], in0=ht[:], scalar=1.0, in1=wb[:],
                op0=mybir.AluOpType.mult, op1=mybir.AluOpType.mult,
                accum_out=logits[:, i:i + 1])

        # sigmoid(logits + bias): the router bias folds into ACT's bias.
        wsig = small.tile([128, ntile], f32, tag="wsig")
        nc.scalar.activation(out=wsig[:], in_=logits[:],
                             func=mybir.ActivationFunctionType.Sigmoid,
                             bias=bb[:])
        nc.sync.dma_start(out=wout[:], in_=wsig[:])

        # ---- pairwise allgather of this core's weights ----
        gin = dram.tile([128, ntile], f32)
        nc.sync.dma_start(out=gin[:], in_=wsig[:])
        gout = dram.tile([1, 2 * tok], f32)
        if fake_gather:
            # single-core timeline-sim stand-in for the pairwise AllGather
            g2 = gout[:].rearrange("a (h t) -> a h t", h=2)
            nc.sync.dma_start(out=g2[:, 0, :], in_=gin.opt())
            nc.sync.dma_start(out=g2[:, 1, :], in_=gin.opt())
        else:
            nc.gpsimd.collective_compute(
                "AllGather",
                mybir.AluOpType.bypass,
                replica_groups=pair_groups,
                ins=[gin.opt()],
                outs=[gout.opt()],
            )
        wall = spool.tile([128, 2 * tok], f32, tag="wall")
        if dma_bcast:
            # replicate the gathered weights to all partitions via a
            # stride-0 DRAM-side read (128 x 32KB descriptors)
            nc.sync.dma_start(out=wall[:],
                              in_=gout[:].broadcast_to((128, 2 * tok)))
        else:
            gs = spool.tile([1, 2 * tok], f32, tag="gs")
            nc.sync.dma_start(out=gs[:], in_=gout[:])
            nc.gpsimd.partition_broadcast(wall[:], gs[:], channels=128)

        # ---- exact kth-largest via 128-way radix bisection on bit space ----
        base = small.tile([128, 1], i32, tag="base0")
        nc.vector.memset(base[:], 0)
        base_alt = small.tile([128, 1], i32, tag="base1")
        cnt = small.tile([128, 1], f32, tag="cnt")
        flag = small.tile([128, 1], f32, tag="flag")
        sumf = small.tile([128, 1], f32, tag="sumf")
        delta = small.tile([128, 1], i32, tag="delta")
        csc = spool.tile([128, 2 * tok], f32, tag="csc")

        for r, s in enumerate(BISECT_STEPS):
            cand = small.tile([128, 1], i32, tag="cand")
            # cand[p] = p * s + base[p].  MUST be on gpsimd: the DVE ALU is
            # fp32-internal, so int32 adds at magnitude 2^30 round to 64s
            # (HW-verified).  Q7 int32 adds are exact.
            nc.gpsimd.tensor_add(cand[:], io[:, r:r + 1], base[:])
            # cnt[p] = sum_j (wall[j] >= float_view(cand[p]))
            # candidate bit patterns are all valid non-negative f32 < 1.0, and
            # the weights are sigmoid outputs in (0,1), so float compare ==
            # bit-int compare (no denormal/negative pitfalls near threshold).
            nc.vector.tensor_scalar(
                out=csc[:], in0=wall[:], scalar1=cand[:].bitcast(f32),
                scalar2=None, op0=mybir.AluOpType.is_ge,
                op1=mybir.AluOpType.add, accum_out=cnt[:])
            # flag[p] = cnt[p] >= k
            nc.vector.tensor_scalar(
                out=flag[:], in0=cnt[:], scalar1=float(k), scalar2=None,
                op0=mybir.AluOpType.is_ge)
            # sumf = sum_p flag[p]  (same value on every partition)
            nc.gpsimd.partition_all_reduce(
                sumf[:], flag[:], channels=128,
                reduce_op=bass_isa.ReduceOp.add)
            # delta = (sumf - 1) * s, computed in f32 (exact: |delta| <=
            # 127 * 2^23 is a 7-bit mantissa times a power of two) with
            # conversion to int32 on the write.
            eng = nc.gpsimd if delta_on_q7 else nc.vector
            eng.tensor_scalar(
                out=delta[:], in0=sumf[:], scalar1=1.0, scalar2=float(s),
                op0=mybir.AluOpType.subtract, op1=mybir.AluOpType.mult)
            # base += delta — gpsimd for exact int32 addition (see above).
            nc.gpsimd.tensor_add(base_alt[:], delta[:], base[:])
            base, base_alt = base_alt, base

        # ---- mask: own weights >= threshold (exact kth-largest value) ----
        mask = small.tile([128, ntile], u8, tag="mask")
        nc.vector.tensor_scalar(
            out=mask[:], in0=wsig[:], scalar1=base[:].bitcast(f32),
            scalar2=None, op0=mybir.AluOpType.is_ge)
        nc.sync.dma_start(out=mout[:], in_=mask[:])

    nc.compile()
    return nc


class Runner:
    """Executes a built Bass module on the 8 axon NeuronCores via PJRT,
    building the sharded jit executable once and reusing it (the stock
    run_bass_kernel_spmd re-jits on every call)."""

    def __init__(self, nc, n_cores=N_CORES):
        import jax
        from jax.sharding import Mesh, PartitionSpec
        from jax.experimental.shard_map import shard_map

        bass2jax.install_neuronx_cc_hook()
        self.n_cores = n_cores
        partition_name = (nc.partition_id_tensor.name
                          if nc.partition_id_tensor else None)
        in_names, out_names, out_avals, zero_outs = [], [], [], []
        for alloc in nc.m.functions[0].allocations:
            if not isinstance(alloc, _mb.MemoryLocationSet):
                continue
            name = alloc.memorylocations[0].name
            if alloc.kind == "ExternalInput":
                if name != partition_name:
                    in_names.append(name)
            elif alloc.kind == "ExternalOutput":
                shape = tuple(alloc.tensor_shape)
                dtype = _mb.dt.np(alloc.dtype)
                out_names.append(name)
                out_avals.append(jax.core.ShapedArray(shape, dtype))
                zero_outs.append(np.zeros(shape, dtype))
        self.in_names, self.out_names = list(in_names), out_names
        self.out_avals, self.zero_outs = out_avals, zero_outs
        n_params, n_outs = len(in_names), len(out_avals)
        self.n_params = n_params
        all_names = in_names + out_names
        if partition_name is not None:
            all_names = all_names + [partition_name]

        def _body(*args):
            operands = list(args)
            if partition_name is not None:
                operands.append(bass2jax.partition_id_tensor())
            return tuple(bass2jax._bass_exec_p.bind(
                *operands,
                out_avals=tuple(out_avals),
                in_names=tuple(all_names),
                out_names=tuple(out_names),
                lowering_input_output_aliases=(),
                sim_require_finite=True,
                sim_require_nnan=True,
                nc=nc,
            ))

        devices = jax.devices()[:n_cores]
        self.mesh = Mesh(np.asarray(devices), ("core",))
        self.pspec = PartitionSpec("core")
        in_specs = (self.pspec,) * (n_params + n_outs)
        out_specs = (self.pspec,) * n_outs
        self.sharded = jax.jit(
            shard_map(_body, mesh=self.mesh, in_specs=in_specs,
                      out_specs=out_specs, check_rep=False),
            donate_argnums=tuple(range(n_params, n_params + n_outs)),
            keep_unused=True)

    def concat_inputs(self, in_maps):
        return [np.concatenate([np.asarray(in_maps[c][nm])
                                for c in range(self.n_cores)], axis=0)
                for nm in self.in_names]

    def fresh_zeros(self):
        return [np.zeros((self.n_cores * z.shape[0], *z.shape[1:]), z.dtype)
                for z in self.zero_outs]

    def call(self, concat_in):
        """concat_in: list of (n_cores*dim0, ...) arrays (host or device)."""
        return self.sharded(*concat_in, *self.fresh_zeros())

    def run(self, in_maps):
        out_arrs = self.call(self.concat_inputs(in_maps))
        return [
            {nm: np.asarray(out_arrs[i]).reshape(
                self.n_cores, *self.out_avals[i].shape)[c]
             for i, nm in enumerate(self.out_names)}
            for c in range(self.n_cores)
        ]


_NC_CACHE = {}


def _get_nc():
    if "full" not in _NC_CACHE:
        _NC_CACHE["full"] = build()
    return _NC_CACHE["full"]


def _get_runner():
    if "runner" not in _NC_CACHE:
        _NC_CACHE["runner"] = Runner(_get_nc())
    return _NC_CACHE["runner"]


def make_in_maps(hidden_states, w, b, n_cores=N_CORES, tok=SEQ // 2):
    hs = np.asarray(hidden_states, dtype=np.float32)
    wv = np.asarray(w, dtype=np.float32).reshape(-1)
    hidden = wv.shape[0]
    w2 = np.ascontiguousarray(np.broadcast_to(wv[None, :], (128, hidden)))
    bias2 = np.full((128, 1), np.float32(b), dtype=np.float32)
    iosc = (np.arange(128, dtype=np.int64)[:, None]
            * np.asarray(BISECT_STEPS, dtype=np.int64)[None, :])
    iosc = iosc.astype(np.int32)
    in_maps = []
    for c in range(n_cores):
        bb, h = c // 2, c % 2
        shard = np.ascontiguousarray(hs[bb, h * tok:(h + 1) * tok, :])
        in_maps.append({"hs": shard, "w2": w2, "bias2": bias2, "iosc": iosc})
    return in_maps


def assemble(results, n_cores=N_CORES, tok=SEQ // 2):
    weights = np.empty((BATCH, SEQ), dtype=np.float32)
    mask = np.empty((BATCH, SEQ), dtype=bool)
    for c in range(n_cores):
        bb, h = c // 2, c % 2
        weights[bb, h * tok:(h + 1) * tok] = results[c]["wout"].reshape(-1)
        mask[bb, h * tok:(h + 1) * tok] = results[c]["mout"].reshape(-1) != 0
    return weights, mask


def kernel(hidden_states, w, b):
    runner = _get_runner()
    in_maps = make_in_maps(hidden_states, w, b)
    return assemble(runner.run(in_maps))
